# revision 30
# baseline (speedup 1.0000x reference)
"""Multi-head attention Trainium2 kernel (B=4, N=2048, D=1024, H=16).

Sharding: 8 cores = 4 batches x 2 head-groups (8 heads each), zero
collectives. Each core:
  - fp16 projections; q,k kept transposed [feat, seq]; v row-layout,
    augmented with a ones column so the PV matmul emits the softmax
    denominator for free (M=65)
  - ramp-up: xT arrives in per-chunk DMA slices and pair-0's k/q project
    first, so the first exps start ~25us in; remaining projections fill
    PE slack under the first units' exps (later q chunks stay lazy as
    mid-attention PE gap filler)
  - attention per head-pair x 512-query chunk: S matmuls packed two
    heads per pass via disjoint PE row groups into one [128,1024] PSUM
    tile, one wide exp on ACT (scale=1/8, fp16 out), PV accumulation
  - normalization: reciprocal_approx_fast on the raw [1,512] denominator
    (DVE), partition_broadcast on GPSIMD, one DVE multiply -- PE and ACT
    stay out of the chain; emission lags two units behind compute
  - out-projection partial [1024,2048] per chunk in fp16, staged via DVE
    (ACT only runs exps); the last chunk's out-projection routes through
    the then-free S-score PSUM banks so its chains pre-run during the
    final softmax normalization
Host sums the two head-group partials per batch in fp32 and adds bias.
Measured: ~389us per core on trn2 (baseline 528us), absmax rel err 8e-4.
"""
from collections import deque
from contextlib import ExitStack

import numpy as np

import concourse.mybir as mybir
import concourse.tile as tile
from concourse import bacc
from concourse.bass_utils import run_bass_kernel_spmd

F32 = mybir.dt.float32
F16 = mybir.dt.float16

P = 128
N = 2048         # sequence length
DI = 1024        # model dim
NH = 8           # heads per core
HD = 64          # head dim
NPAIR = 4        # head pairs per core
KT = 8           # contraction tiles for projections
CH = 512         # query chunk width
NCHUNK = 4       # chunks per sequence
MT = 16          # key tiles (m) per sequence
ET = 8           # output-feature blocks
SCALE = HD ** -0.5

_NC_CACHE = None


def _build():
    nc = bacc.Bacc("TRN2", target_bir_lowering=False, debug=False)

    xT = nc.dram_tensor("xT", [KT, NCHUNK, P, CH], F16,
                        kind="ExternalInput").ap()
    wqkA = nc.dram_tensor("wqkA", [8, P, KT, P], F16, kind="ExternalInput").ap()
    wvA = nc.dram_tensor("wvA", [P, KT, 512], F16, kind="ExternalInput").ap()
    woT = nc.dram_tensor("woT", [512, DI], F16, kind="ExternalInput").ap()
    outT = nc.dram_tensor("outT", [DI, N], F16, kind="ExternalOutput").ap()

    woT_r = woT.rearrange("(k p) e -> k p e", p=P)      # [4, 128, 1024]
    outT_r = outT.rearrange("(e p) n -> e p n", p=P)    # [8, 128, 2048]

    with tile.TileContext(nc) as tc, ExitStack() as persist:
        qk_pool = persist.enter_context(tc.tile_pool(name="qkp", bufs=8))
        va_pool = persist.enter_context(tc.tile_pool(name="vap", bufs=1))
        misc = persist.enter_context(tc.tile_pool(name="misc", bufs=1))
        xt_pool = persist.enter_context(tc.tile_pool(name="xt", bufs=8))
        wq_pool = persist.enter_context(tc.tile_pool(name="wq", bufs=4))
        wv_pool = persist.enter_context(tc.tile_pool(name="wv", bufs=1))
        wo_pool = persist.enter_context(tc.tile_pool(name="wo", bufs=4))
        exp_pool = persist.enter_context(tc.tile_pool(name="expp", bufs=17))
        ot_pool = persist.enter_context(tc.tile_pool(name="ot", bufs=8))
        osb_pool = persist.enter_context(tc.tile_pool(name="osb", bufs=8))
        stage_pool = persist.enter_context(tc.tile_pool(name="stg", bufs=3))
        rden_pool = persist.enter_context(tc.tile_pool(name="rden", bufs=8))
        stg2_pool = persist.enter_context(tc.tile_pool(name="stg2", bufs=2))
        rbc_pool = persist.enter_context(tc.tile_pool(name="rbc", bufs=4))
        sps_pool = persist.enter_context(
            tc.tile_pool(name="sps", bufs=2, space="PSUM"))
        oaug_pool = persist.enter_context(
            tc.tile_pool(name="oaug", bufs=2, space="PSUM"))
        aux_pool = persist.enter_context(
            tc.tile_pool(name="aux", bufs=2, space="PSUM"))

        # ---- DMAs in priority order: pair-0 weights first, then x ----
        # xT arrives in per-chunk column slices so the first kproj/qproj
        # chains (and with them the first exps) start ~6us in instead of
        # waiting for the whole 4MB transfer
        wqk_k = [None] * 8
        wqk_k[4] = wq_pool.tile([P, KT, P], F16, name="wk4", tag="wk")
        nc.sync.dma_start(wqk_k[4][:], wqkA[4])
        xt = [xt_pool.tile([P, N], F16, name=f"xt{k}", tag="xt")
              for k in range(KT)]
        for k in range(KT):
            nc.sync.dma_start(xt[k][:, 0:CH], xT[k, 0])
        wq = [wq_pool.tile([P, KT, P], F16, name=f"wq{f}", tag="wq")
              for f in range(4)]
        nc.sync.dma_start(wq[0][:], wqkA[0])
        for k in range(KT):
            nc.sync.dma_start(xt[k][:, CH:2 * CH], xT[k, 1])
        nc.sync.dma_start(wq[1][:], wqkA[1])
        wqk_k[5] = wq_pool.tile([P, KT, P], F16, name="wk5", tag="wk")
        nc.sync.dma_start(wqk_k[5][:], wqkA[5])
        nc.sync.dma_start(wq[2][:], wqkA[2])
        nc.sync.dma_start(wq[3][:], wqkA[3])
        for c in (2, 3):
            for k in range(KT):
                nc.sync.dma_start(xt[k][:, c * CH:(c + 1) * CH], xT[k, c])
        for f in (6, 7):
            wqk_k[f] = wq_pool.tile([P, KT, P], F16, name=f"wk{f}", tag="wk")
            nc.sync.dma_start(wqk_k[f][:], wqkA[f])
        wv = wv_pool.tile([P, KT, 512], F16)
        nc.sync.dma_start(wv[:], wvA[:])
        wo = [wo_pool.tile([P, DI], F16, name=f"wo{kk}", tag="wo")
              for kk in range(NPAIR)]
        for kk in range(NPAIR):
            nc.sync.dma_start(wo[kk][:], woT_r[kk])

        # qkT tiles: 0..3 = q head-pairs, 4..7 = k head-pairs.
        # Tile j holds heads 2j (parts 0:64) and 2j+1 (parts 64:128).
        qkT = [qk_pool.tile([P, N], F16, name=f"qkT{t}", tag="qkT")
               for t in range(8)]
        v_aug = va_pool.tile([P, MT, NH, HD + 1], F16)
        nc.vector.memset(v_aug[:, :, :, HD:HD + 1], 1.0)

        # ---------------- projection emitters --------------------------
        def emit_kproj(f, chunks=tuple(range(NCHUNK))):
            for c in chunks:
                ps = aux_pool.tile([P, CH], F32, tag="aux", name=f"kp_{f}_{c}")
                for k in range(KT):
                    nc.tensor.matmul(ps[:], wqk_k[f][:, k, :],
                                     xt[k][:, c * CH:(c + 1) * CH],
                                     start=(k == 0), stop=(k == KT - 1))
                nc.vector.tensor_copy(qkT[f][:, c * CH:(c + 1) * CH], ps[:])

        def emit_vproj_row(r):
            ps = aux_pool.tile([P, CH], F32, tag="aux", name=f"vp_{r}")
            for k in range(KT):
                nc.tensor.matmul(ps[:], xt[k][:, r * P:(r + 1) * P],
                                 wv[:, k, :],
                                 start=(k == 0), stop=(k == KT - 1))
            nc.vector.tensor_copy(v_aug[:, r, :, 0:HD],
                                  ps.rearrange("p (h d) -> p h d", d=HD))

        def emit_qproj(c, f):
            csl = slice(c * CH, (c + 1) * CH)
            ps = aux_pool.tile([P, CH], F32, tag="aux", name=f"qp_{c}_{f}")
            for k in range(KT):
                nc.tensor.matmul(ps[:], wq[f][:, k, :], xt[k][:, csl],
                                 start=(k == 0), stop=(k == KT - 1))
            nc.vector.tensor_copy(qkT[f][:, csl], ps[:])

        # ---------------- attention emitters ---------------------------
        ot_map = {}

        def emit_sphase(c, p):
            csl = slice(c * CH, (c + 1) * CH)
            qA = qkT[p][0:HD, csl]
            qB = qkT[p][HD:P, csl]
            kTl = qkT[4 + p]
            expPs = []
            for m in range(MT):
                msl = slice(m * P, (m + 1) * P)
                s_ps = sps_pool.tile([P, 2 * CH], F32, tag="sps",
                                     name=f"sps_{c}_{p}_{m}")
                # packed S matmuls: head A rows 0:64, head B rows 64:128
                # (disjoint PE row groups, run concurrently)
                nc.tensor.matmul(s_ps[:, 0:CH], kTl[0:HD, msl], qA,
                                 start=True, stop=True)
                nc.tensor.matmul(s_ps[:, CH:2 * CH], kTl[HD:P, msl], qB,
                                 start=True, stop=True)
                expP = exp_pool.tile([P, 2 * CH], F16, tag="expp",
                                     name=f"expP_{c}_{p}_{m}")
                nc.scalar.activation(expP[:], s_ps[:],
                                     mybir.ActivationFunctionType.Exp,
                                     scale=SCALE)
                expPs.append(expP)
            return expPs

        def emit_pvphase(c, p, expPs, per_m=None):
            oaugA = oaug_pool.tile([P, CH], F32, tag="oaug",
                                   name=f"oaugA_{c}_{p}")
            oaugB = oaug_pool.tile([P, CH], F32, tag="oaug",
                                   name=f"oaugB_{c}_{p}")
            for m in range(MT):
                if per_m is not None:
                    per_m(m)
                nc.tensor.matmul(oaugA[0:HD + 1, :],
                                 v_aug[:, m, 2 * p, :],
                                 expPs[m][:, 0:CH],
                                 start=(m == 0), stop=(m == MT - 1))
                nc.tensor.matmul(oaugB[0:HD + 1, :],
                                 v_aug[:, m, 2 * p + 1, :],
                                 expPs[m][:, CH:2 * CH],
                                 start=(m == 0), stop=(m == MT - 1))

            # evacuate + fast reciprocal of the raw [1,512] denominator row
            # (den copied to a partition-0 tile: reciprocal_approx_fast
            # cannot read from a partition-offset AP)
            o_sbs = []
            last = (c, p) == (NCHUNK - 1, NPAIR - 1)
            for half, oaug in ((0, oaugA), (1, oaugB)):
                den = rden_pool.tile([1, CH], F32, tag="rden",
                                     name=f"den_{c}_{p}_{half}")
                nc.vector.tensor_copy(den[:], oaug[HD:HD + 1, :])
                rden = rden_pool.tile([1, CH], F32, tag="rden",
                                      name=f"rden_{c}_{p}_{half}")
                nc.vector.reciprocal_approx_fast(rden[:], den[:])
                if last:
                    # tail: normalize straight from the PSUM bank
                    o_sb = oaug
                else:
                    o_sb = osb_pool.tile([HD, CH], F16, tag="osb",
                                         name=f"osb_{c}_{p}_{half}")
                    nc.vector.tensor_copy(o_sb[:], oaug[0:HD, :])
                o_sbs.append((o_sb, rden))
            return (c, p, o_sbs)

        def emit_unit(c, p):
            return emit_pvphase(c, p, emit_sphase(c, p))

        def emit_norm(unit):
            # broadcast the reciprocal across partitions on GPSIMD, then
            # one DVE multiply; emitted one unit late so nothing waits
            c, p, o_sbs = unit
            ot_p = ot_pool.tile([P, CH], F16, name=f"ot_{c}_{p}", tag="ot")
            for half, (o_sb, rden) in ((0, o_sbs[0]), (1, o_sbs[1])):
                rbc = rbc_pool.tile([HD, CH], F32, tag="rbc",
                                    name=f"rbc_{c}_{p}_{half}")
                nc.gpsimd.partition_broadcast(rbc[:], rden[:], channels=HD)
                nc.vector.tensor_tensor(
                    ot_p[half * HD:(half + 1) * HD, :],
                    o_sb[0:HD, :], rbc[:], mybir.AluOpType.mult)
            ot_map[(c, p)] = ot_p

        def emit_outproj(c, es=tuple(range(ET))):
            csl = slice(c * CH, (c + 1) * CH)
            for e in es:
                pso = aux_pool.tile([P, CH], F32, tag="aux",
                                    name=f"pso_{c}_{e}")
                for p in range(NPAIR):
                    nc.tensor.matmul(pso[:],
                                     wo[p][:, e * P:(e + 1) * P],
                                     ot_map[(c, p)][:],
                                     start=(p == 0), stop=(p == NPAIR - 1))
                st = stage_pool.tile([P, CH], F16, tag="stg",
                                     name=f"st_{c}_{e}")
                nc.vector.tensor_copy(st[:], pso[:])
                nc.sync.dma_start(outT_r[e][:, csl], st[:])

        def emit_outproj_final(c):
            # last chunk: the sps banks are free (no exps left) — pack two
            # e-blocks per [128,1024] PSUM tile so four chains pre-run
            # their p<3 matmuls while the last norm is still in flight
            csl = slice(c * CH, (c + 1) * CH)
            for ep in range(ET // 2):
                pso = sps_pool.tile([P, 2 * CH], F32, tag="sps",
                                    name=f"psof_{ep}")
                for half in (0, 1):
                    e = 2 * ep + half
                    for p in range(NPAIR):
                        nc.tensor.matmul(pso[:, half * CH:(half + 1) * CH],
                                         wo[p][:, e * P:(e + 1) * P],
                                         ot_map[(c, p)][:],
                                         start=(p == 0),
                                         stop=(p == NPAIR - 1))
                st = stg2_pool.tile([P, 2 * CH], F16, tag="stg2",
                                    name=f"stf_{ep}")
                nc.vector.tensor_copy(st[:, 0:CH], pso[:, 0:CH])
                nc.sync.dma_start(outT_r[2 * ep][:, csl], st[:, 0:CH])
                nc.vector.tensor_copy(st[:, CH:2 * CH], pso[:, CH:2 * CH])
                nc.scalar.dma_start(outT_r[2 * ep + 1][:, csl],
                                    st[:, CH:2 * CH])

        # ---------------- emission schedule -----------------------------
        # Pair-0 projections first so unit (0,0) unblocks ~35us in; the
        # rest of the projections fill PE slack under the first exps.
        # Software pipeline: the norm for unit i is emitted after unit
        # i+1's matmuls, and chunk c's out-projection after chunk c+1's
        # second unit, so the PE never waits on the DVE chain.
        pend = deque()
        # chunk-0 ramp-up: pair-0's S-phase starts as soon as kT4-chunk0
        # and q pair-0 land; vproj rows interleave with PV(0,0) matmuls
        # (PV(0,0,m) must follow vproj row m in program order) so the PE
        # alternates instead of draining all of vproj before any PV
        emit_kproj(4, chunks=(0,))
        emit_qproj(0, 0)
        emit_qproj(0, 1)
        emit_kproj(4, chunks=(1, 2, 3))
        sp00 = emit_sphase(0, 0)
        emit_kproj(5, chunks=(0,))
        emit_kproj(5, chunks=(1, 2, 3))
        sp01 = emit_sphase(0, 1)
        for r in range(MT):
            emit_vproj_row(r)
        pend.append(emit_pvphase(0, 0, sp00))
        emit_qproj(0, 2)
        emit_qproj(0, 3)
        pend.append(emit_pvphase(0, 1, sp01))
        emit_qproj(1, 0)
        emit_qproj(1, 1)
        emit_kproj(6)
        emit_kproj(7)
        emit_qproj(1, 2)
        emit_qproj(1, 3)
        for p in (2, 3):
            pend.append(emit_unit(0, p))
            if len(pend) > 2:
                emit_norm(pend.popleft())
        for c in range(1, NCHUNK):
            for p in range(NPAIR):
                pend.append(emit_unit(c, p))
                lag = 1 if c == NCHUNK - 1 else 2
                while len(pend) > lag:
                    emit_norm(pend.popleft())
                if c + 1 < NCHUNK and p in (0, 1):
                    emit_qproj(c + 1, 2 * p)
                    emit_qproj(c + 1, 2 * p + 1)
                if p == 2:
                    while pend and pend[0][0] < c:
                        emit_norm(pend.popleft())
                    emit_outproj(c - 1, es=(0, 1, 2, 3))
                if p == 3:
                    emit_outproj(c - 1, es=(4, 5, 6, 7))
        while pend:
            emit_norm(pend.popleft())
        emit_outproj_final(NCHUNK - 1)

    nc.compile()
    return nc


def _get_nc():
    global _NC_CACHE
    if _NC_CACHE is None:
        _NC_CACHE = _build()
    return _NC_CACHE


def _make_in_maps(x, w_qkv, w_out):
    per_g = []
    for g in range(2):
        qk_g = np.concatenate([w_qkv[g * 512:(g + 1) * 512],
                               w_qkv[DI + g * 512:DI + (g + 1) * 512]], axis=0)
        wqkT = np.ascontiguousarray(qk_g.T)               # [1024 d, 1024 f]
        wqkA = np.ascontiguousarray(
            wqkT.reshape(KT, P, 8, P).transpose(2, 1, 0, 3).astype(np.float16))
        v_g = w_qkv[2 * DI + g * 512:2 * DI + (g + 1) * 512]
        wvT = np.ascontiguousarray(v_g.T)                 # [1024 d, 512 f]
        wvA = np.ascontiguousarray(
            wvT.reshape(KT, P, 512).transpose(1, 0, 2).astype(np.float16))
        woTg = np.ascontiguousarray(
            w_out[:, g * 512:(g + 1) * 512].T.astype(np.float16))
        per_g.append((wqkA, wvA, woTg))

    in_maps = []
    for c in range(8):
        b, g = c // 2, c % 2
        wqkA, wvA, woTg = per_g[g]
        in_maps.append({
            "xT": np.ascontiguousarray(
                x[b].T.astype(np.float16).reshape(KT, P, NCHUNK, CH)
                .transpose(0, 2, 1, 3)),
            "wqkA": wqkA,
            "wvA": wvA,
            "woT": woTg,
        })
    return in_maps


def kernel(x, w_qkv, w_out, b_out):
    x = np.asarray(x, dtype=np.float32)
    w_qkv = np.asarray(w_qkv, dtype=np.float32)
    w_out = np.asarray(w_out, dtype=np.float32)
    b_out = np.asarray(b_out, dtype=np.float32)
    B = x.shape[0]

    in_maps = _make_in_maps(x, w_qkv, w_out)
    nc = _get_nc()
    res = run_bass_kernel_spmd(nc, in_maps, core_ids=list(range(8)))
    parts = [r["outT"] for r in res.results]
    out = np.empty((B, N, DI), dtype=np.float32)
    for b in range(B):
        out[b] = (parts[2 * b].astype(np.float32)
                  + parts[2 * b + 1].astype(np.float32)).T + b_out
    return out


# revision 31
# speedup vs baseline: 1.0068x; 1.0068x over previous
"""Multi-head attention Trainium2 kernel (B=4, N=2048, D=1024, H=16).

Sharding: 8 cores = 4 batches x 2 head-groups (8 heads each), zero
collectives. Each core:
  - fp16 projections; q,k kept transposed [feat, seq]; v row-layout,
    augmented with a ones column so the PV matmul emits the softmax
    denominator for free (M=65)
  - ramp-up: xT arrives in per-chunk DMA slices and pair-0's k/q project
    first, so the first exps start ~25us in; remaining projections fill
    PE slack under the first units' exps (later q chunks stay lazy as
    mid-attention PE gap filler)
  - attention per head-pair x 512-query chunk: S matmuls packed two
    heads per pass via disjoint PE row groups into one [128,1024] PSUM
    tile, one wide exp on ACT (scale=1/8, fp16 out), PV accumulation
  - normalization: reciprocal_approx_fast on the raw [1,512] denominator
    (DVE), partition_broadcast on GPSIMD, one DVE multiply -- PE and ACT
    stay out of the chain; emission lags two units behind compute
  - out-projection partial [1024,2048] per chunk in fp16, staged via DVE
    (ACT only runs exps); the last chunk's out-projection routes through
    the then-free S-score PSUM banks so its chains pre-run during the
    final softmax normalization
Host sums the two head-group partials per batch in fp32 and adds bias.
Measured: ~389us per core on trn2 (baseline 528us), absmax rel err 8e-4.
"""
from collections import deque
from contextlib import ExitStack

import numpy as np

import concourse.mybir as mybir
import concourse.tile as tile
from concourse import bacc
from concourse.bass_utils import run_bass_kernel_spmd

F32 = mybir.dt.float32
F16 = mybir.dt.float16

P = 128
N = 2048         # sequence length
DI = 1024        # model dim
NH = 8           # heads per core
HD = 64          # head dim
NPAIR = 4        # head pairs per core
KT = 8           # contraction tiles for projections
CH = 512         # query chunk width
NCHUNK = 4       # chunks per sequence
MT = 16          # key tiles (m) per sequence
ET = 8           # output-feature blocks
SCALE = HD ** -0.5

_NC_CACHE = None


def _build():
    nc = bacc.Bacc("TRN2", target_bir_lowering=False, debug=False)

    xT = nc.dram_tensor("xT", [KT, NCHUNK, P, CH], F16,
                        kind="ExternalInput").ap()
    wqkA = nc.dram_tensor("wqkA", [8, P, KT, P], F16, kind="ExternalInput").ap()
    wvA = nc.dram_tensor("wvA", [P, KT, 512], F16, kind="ExternalInput").ap()
    woT = nc.dram_tensor("woT", [512, DI], F16, kind="ExternalInput").ap()
    outT = nc.dram_tensor("outT", [DI, N], F16, kind="ExternalOutput").ap()

    woT_r = woT.rearrange("(k p) e -> k p e", p=P)      # [4, 128, 1024]
    outT_r = outT.rearrange("(e p) n -> e p n", p=P)    # [8, 128, 2048]

    with tile.TileContext(nc) as tc, ExitStack() as persist:
        qk_pool = persist.enter_context(tc.tile_pool(name="qkp", bufs=8))
        va_pool = persist.enter_context(tc.tile_pool(name="vap", bufs=1))
        misc = persist.enter_context(tc.tile_pool(name="misc", bufs=1))
        xt_pool = persist.enter_context(tc.tile_pool(name="xt", bufs=8))
        wq_pool = persist.enter_context(tc.tile_pool(name="wq", bufs=4))
        wv_pool = persist.enter_context(tc.tile_pool(name="wv", bufs=1))
        wo_pool = persist.enter_context(tc.tile_pool(name="wo", bufs=4))
        exp_pool = persist.enter_context(tc.tile_pool(name="expp", bufs=17))
        ot_pool = persist.enter_context(tc.tile_pool(name="ot", bufs=8))
        osb_pool = persist.enter_context(tc.tile_pool(name="osb", bufs=8))
        stage_pool = persist.enter_context(tc.tile_pool(name="stg", bufs=3))
        rden_pool = persist.enter_context(tc.tile_pool(name="rden", bufs=12))
        stg2_pool = persist.enter_context(tc.tile_pool(name="stg2", bufs=2))
        rbc_pool = persist.enter_context(tc.tile_pool(name="rbc", bufs=4))
        sps_pool = persist.enter_context(
            tc.tile_pool(name="sps", bufs=2, space="PSUM"))
        oaug_pool = persist.enter_context(
            tc.tile_pool(name="oaug", bufs=2, space="PSUM"))
        aux_pool = persist.enter_context(
            tc.tile_pool(name="aux", bufs=2, space="PSUM"))

        # ---- DMAs in priority order: pair-0 weights first, then x ----
        # xT arrives in per-chunk column slices so the first kproj/qproj
        # chains (and with them the first exps) start ~6us in instead of
        # waiting for the whole 4MB transfer
        wqk_k = [None] * 8
        wqk_k[4] = wq_pool.tile([P, KT, P], F16, name="wk4", tag="wk")
        nc.sync.dma_start(wqk_k[4][:], wqkA[4])
        xt = [xt_pool.tile([P, N], F16, name=f"xt{k}", tag="xt")
              for k in range(KT)]
        for k in range(KT):
            nc.sync.dma_start(xt[k][:, 0:CH], xT[k, 0])
        wq = [wq_pool.tile([P, KT, P], F16, name=f"wq{f}", tag="wq")
              for f in range(4)]
        nc.sync.dma_start(wq[0][:], wqkA[0])
        for k in range(KT):
            nc.sync.dma_start(xt[k][:, CH:2 * CH], xT[k, 1])
        nc.sync.dma_start(wq[1][:], wqkA[1])
        wqk_k[5] = wq_pool.tile([P, KT, P], F16, name="wk5", tag="wk")
        nc.sync.dma_start(wqk_k[5][:], wqkA[5])
        nc.sync.dma_start(wq[2][:], wqkA[2])
        nc.sync.dma_start(wq[3][:], wqkA[3])
        for c in (2, 3):
            for k in range(KT):
                nc.sync.dma_start(xt[k][:, c * CH:(c + 1) * CH], xT[k, c])
        for f in (6, 7):
            wqk_k[f] = wq_pool.tile([P, KT, P], F16, name=f"wk{f}", tag="wk")
            nc.sync.dma_start(wqk_k[f][:], wqkA[f])
        wv = wv_pool.tile([P, KT, 512], F16)
        nc.sync.dma_start(wv[:], wvA[:])
        wo = [wo_pool.tile([P, DI], F16, name=f"wo{kk}", tag="wo")
              for kk in range(NPAIR)]
        for kk in range(NPAIR):
            nc.sync.dma_start(wo[kk][:], woT_r[kk])

        # qkT tiles: 0..3 = q head-pairs, 4..7 = k head-pairs.
        # Tile j holds heads 2j (parts 0:64) and 2j+1 (parts 64:128).
        qkT = [qk_pool.tile([P, N], F16, name=f"qkT{t}", tag="qkT")
               for t in range(8)]
        v_aug = va_pool.tile([P, MT, NH, HD + 1], F16)
        nc.vector.memset(v_aug[:, :, :, HD:HD + 1], 1.0)

        # ---------------- projection emitters --------------------------
        def emit_kproj(f, chunks=tuple(range(NCHUNK))):
            for c in chunks:
                ps = aux_pool.tile([P, CH], F32, tag="aux", name=f"kp_{f}_{c}")
                for k in range(KT):
                    nc.tensor.matmul(ps[:], wqk_k[f][:, k, :],
                                     xt[k][:, c * CH:(c + 1) * CH],
                                     start=(k == 0), stop=(k == KT - 1))
                nc.vector.tensor_copy(qkT[f][:, c * CH:(c + 1) * CH], ps[:])

        def emit_vproj_row(r):
            ps = aux_pool.tile([P, CH], F32, tag="aux", name=f"vp_{r}")
            for k in range(KT):
                nc.tensor.matmul(ps[:], xt[k][:, r * P:(r + 1) * P],
                                 wv[:, k, :],
                                 start=(k == 0), stop=(k == KT - 1))
            nc.vector.tensor_copy(v_aug[:, r, :, 0:HD],
                                  ps.rearrange("p (h d) -> p h d", d=HD))

        def emit_qproj(c, f):
            csl = slice(c * CH, (c + 1) * CH)
            ps = aux_pool.tile([P, CH], F32, tag="aux", name=f"qp_{c}_{f}")
            for k in range(KT):
                nc.tensor.matmul(ps[:], wq[f][:, k, :], xt[k][:, csl],
                                 start=(k == 0), stop=(k == KT - 1))
            nc.vector.tensor_copy(qkT[f][:, csl], ps[:])

        # ---------------- attention emitters ---------------------------
        ot_map = {}

        def emit_sphase(c, p):
            csl = slice(c * CH, (c + 1) * CH)
            qA = qkT[p][0:HD, csl]
            qB = qkT[p][HD:P, csl]
            kTl = qkT[4 + p]
            expPs = []
            for m in range(MT):
                msl = slice(m * P, (m + 1) * P)
                s_ps = sps_pool.tile([P, 2 * CH], F32, tag="sps",
                                     name=f"sps_{c}_{p}_{m}")
                # packed S matmuls: head A rows 0:64, head B rows 64:128
                # (disjoint PE row groups, run concurrently)
                nc.tensor.matmul(s_ps[:, 0:CH], kTl[0:HD, msl], qA,
                                 start=True, stop=True)
                nc.tensor.matmul(s_ps[:, CH:2 * CH], kTl[HD:P, msl], qB,
                                 start=True, stop=True)
                expP = exp_pool.tile([P, 2 * CH], F16, tag="expp",
                                     name=f"expP_{c}_{p}_{m}")
                nc.scalar.activation(expP[:], s_ps[:],
                                     mybir.ActivationFunctionType.Exp,
                                     scale=SCALE)
                expPs.append(expP)
            return expPs

        def emit_pvphase(c, p, expPs, per_m=None):
            oaugA = oaug_pool.tile([P, CH], F32, tag="oaug",
                                   name=f"oaugA_{c}_{p}")
            oaugB = oaug_pool.tile([P, CH], F32, tag="oaug",
                                   name=f"oaugB_{c}_{p}")
            for m in range(MT):
                if per_m is not None:
                    per_m(m)
                nc.tensor.matmul(oaugA[0:HD + 1, :],
                                 v_aug[:, m, 2 * p, :],
                                 expPs[m][:, 0:CH],
                                 start=(m == 0), stop=(m == MT - 1))
                nc.tensor.matmul(oaugB[0:HD + 1, :],
                                 v_aug[:, m, 2 * p + 1, :],
                                 expPs[m][:, CH:2 * CH],
                                 start=(m == 0), stop=(m == MT - 1))

            # evacuate + fast reciprocal of the raw [1,512] denominator row
            # (den copied to a partition-0 tile: reciprocal_approx_fast
            # cannot read from a partition-offset AP)
            o_sbs = []
            last = (c, p) == (NCHUNK - 1, NPAIR - 1)
            for half, oaug in ((0, oaugA), (1, oaugB)):
                den = rden_pool.tile([1, CH], F32, tag="rden",
                                     name=f"den_{c}_{p}_{half}")
                nc.vector.tensor_copy(den[:], oaug[HD:HD + 1, :])
                rden = rden_pool.tile([1, CH], F32, tag="rden",
                                      name=f"rden_{c}_{p}_{half}")
                nc.vector.reciprocal_approx_fast(rden[:], den[:])
                if last:
                    # tail: normalize straight from the PSUM bank
                    o_sb = oaug
                else:
                    o_sb = osb_pool.tile([HD, CH], F16, tag="osb",
                                         name=f"osb_{c}_{p}_{half}")
                    nc.vector.tensor_copy(o_sb[:], oaug[0:HD, :])
                o_sbs.append((o_sb, rden))
            return (c, p, o_sbs)

        def emit_unit(c, p):
            return emit_pvphase(c, p, emit_sphase(c, p))

        def emit_norm(unit):
            # broadcast the reciprocal across partitions on GPSIMD, then
            # one DVE multiply; emitted one unit late so nothing waits
            c, p, o_sbs = unit
            ot_p = ot_pool.tile([P, CH], F16, name=f"ot_{c}_{p}", tag="ot")
            for half, (o_sb, rden) in ((0, o_sbs[0]), (1, o_sbs[1])):
                rbc = rbc_pool.tile([HD, CH], F32, tag="rbc",
                                    name=f"rbc_{c}_{p}_{half}")
                nc.gpsimd.partition_broadcast(rbc[:], rden[:], channels=HD)
                nc.vector.tensor_tensor(
                    ot_p[half * HD:(half + 1) * HD, :],
                    o_sb[0:HD, :], rbc[:], mybir.AluOpType.mult)
            ot_map[(c, p)] = ot_p

        def emit_outproj(c, es=tuple(range(ET))):
            csl = slice(c * CH, (c + 1) * CH)
            for e in es:
                pso = aux_pool.tile([P, CH], F32, tag="aux",
                                    name=f"pso_{c}_{e}")
                for p in range(NPAIR):
                    nc.tensor.matmul(pso[:],
                                     wo[p][:, e * P:(e + 1) * P],
                                     ot_map[(c, p)][:],
                                     start=(p == 0), stop=(p == NPAIR - 1))
                st = stage_pool.tile([P, CH], F16, tag="stg",
                                     name=f"st_{c}_{e}")
                nc.vector.tensor_copy(st[:], pso[:])
                nc.sync.dma_start(outT_r[e][:, csl], st[:])

        def emit_outproj_final(c):
            # last chunk: the sps banks are free (no exps left) — pack two
            # e-blocks per [128,1024] PSUM tile so four chains pre-run
            # their p<3 matmuls while the last norm is still in flight
            csl = slice(c * CH, (c + 1) * CH)
            for ep in range(ET // 2):
                pso = sps_pool.tile([P, 2 * CH], F32, tag="sps",
                                    name=f"psof_{ep}")
                for half in (0, 1):
                    e = 2 * ep + half
                    for p in range(NPAIR):
                        nc.tensor.matmul(pso[:, half * CH:(half + 1) * CH],
                                         wo[p][:, e * P:(e + 1) * P],
                                         ot_map[(c, p)][:],
                                         start=(p == 0),
                                         stop=(p == NPAIR - 1))
                st = stg2_pool.tile([P, 2 * CH], F16, tag="stg2",
                                    name=f"stf_{ep}")
                nc.vector.tensor_copy(st[:, 0:CH], pso[:, 0:CH])
                nc.sync.dma_start(outT_r[2 * ep][:, csl], st[:, 0:CH])
                nc.vector.tensor_copy(st[:, CH:2 * CH], pso[:, CH:2 * CH])
                nc.scalar.dma_start(outT_r[2 * ep + 1][:, csl],
                                    st[:, CH:2 * CH])

        # ---------------- emission schedule -----------------------------
        # Pair-0 projections first so unit (0,0) unblocks ~35us in; the
        # rest of the projections fill PE slack under the first exps.
        # Software pipeline: the norm for unit i is emitted after unit
        # i+1's matmuls, and chunk c's out-projection after chunk c+1's
        # second unit, so the PE never waits on the DVE chain.
        pend = deque()
        # chunk-0 ramp-up: pair-0's S-phase starts as soon as kT4-chunk0
        # and q pair-0 land; vproj rows interleave with PV(0,0) matmuls
        # (PV(0,0,m) must follow vproj row m in program order) so the PE
        # alternates instead of draining all of vproj before any PV
        emit_kproj(4, chunks=(0,))
        emit_qproj(0, 0)
        emit_qproj(0, 1)
        emit_kproj(4, chunks=(1, 2, 3))
        sp00 = emit_sphase(0, 0)
        emit_kproj(5, chunks=(0,))
        emit_kproj(5, chunks=(1, 2, 3))
        sp01 = emit_sphase(0, 1)
        for r in range(MT):
            emit_vproj_row(r)
        pend.append(emit_pvphase(0, 0, sp00))
        emit_qproj(0, 2)
        emit_qproj(0, 3)
        pend.append(emit_pvphase(0, 1, sp01))
        emit_qproj(1, 0)
        emit_qproj(1, 1)
        emit_kproj(6)
        emit_kproj(7)
        emit_qproj(1, 2)
        emit_qproj(1, 3)
        for p in (2, 3):
            pend.append(emit_unit(0, p))
            if len(pend) > 2:
                emit_norm(pend.popleft())
        for c in range(1, NCHUNK):
            for p in range(NPAIR):
                pend.append(emit_unit(c, p))
                lag = 1 if c == NCHUNK - 1 else 2
                while len(pend) > lag:
                    emit_norm(pend.popleft())
                if c + 1 < NCHUNK and p in (0, 1):
                    emit_qproj(c + 1, 2 * p)
                    emit_qproj(c + 1, 2 * p + 1)
                if p == 2:
                    while pend and pend[0][0] < c:
                        emit_norm(pend.popleft())
                    emit_outproj(c - 1, es=(0, 1, 2, 3))
                if p == 3:
                    emit_outproj(c - 1, es=(4, 5, 6, 7))
        while pend:
            emit_norm(pend.popleft())
        emit_outproj_final(NCHUNK - 1)

    nc.compile()
    return nc


def _get_nc():
    global _NC_CACHE
    if _NC_CACHE is None:
        _NC_CACHE = _build()
    return _NC_CACHE


def _make_in_maps(x, w_qkv, w_out):
    per_g = []
    for g in range(2):
        qk_g = np.concatenate([w_qkv[g * 512:(g + 1) * 512],
                               w_qkv[DI + g * 512:DI + (g + 1) * 512]], axis=0)
        wqkT = np.ascontiguousarray(qk_g.T)               # [1024 d, 1024 f]
        wqkA = np.ascontiguousarray(
            wqkT.reshape(KT, P, 8, P).transpose(2, 1, 0, 3).astype(np.float16))
        v_g = w_qkv[2 * DI + g * 512:2 * DI + (g + 1) * 512]
        wvT = np.ascontiguousarray(v_g.T)                 # [1024 d, 512 f]
        wvA = np.ascontiguousarray(
            wvT.reshape(KT, P, 512).transpose(1, 0, 2).astype(np.float16))
        woTg = np.ascontiguousarray(
            w_out[:, g * 512:(g + 1) * 512].T.astype(np.float16))
        per_g.append((wqkA, wvA, woTg))

    in_maps = []
    for c in range(8):
        b, g = c // 2, c % 2
        wqkA, wvA, woTg = per_g[g]
        in_maps.append({
            "xT": np.ascontiguousarray(
                x[b].T.astype(np.float16).reshape(KT, P, NCHUNK, CH)
                .transpose(0, 2, 1, 3)),
            "wqkA": wqkA,
            "wvA": wvA,
            "woT": woTg,
        })
    return in_maps


def kernel(x, w_qkv, w_out, b_out):
    x = np.asarray(x, dtype=np.float32)
    w_qkv = np.asarray(w_qkv, dtype=np.float32)
    w_out = np.asarray(w_out, dtype=np.float32)
    b_out = np.asarray(b_out, dtype=np.float32)
    B = x.shape[0]

    in_maps = _make_in_maps(x, w_qkv, w_out)
    nc = _get_nc()
    res = run_bass_kernel_spmd(nc, in_maps, core_ids=list(range(8)))
    parts = [r["outT"] for r in res.results]
    out = np.empty((B, N, DI), dtype=np.float32)
    for b in range(B):
        out[b] = (parts[2 * b].astype(np.float32)
                  + parts[2 * b + 1].astype(np.float32)).T + b_out
    return out


# revision 32
# speedup vs baseline: 1.0079x; 1.0011x over previous
"""Multi-head attention Trainium2 kernel (B=4, N=2048, D=1024, H=16).

Sharding: 8 cores = 4 batches x 2 head-groups (8 heads each), zero
collectives. Each core:
  - fp16 projections; q,k kept transposed [feat, seq]; v row-layout,
    augmented with a ones column so the PV matmul emits the softmax
    denominator for free (M=65)
  - ramp-up: xT arrives in per-chunk DMA slices and pair-0's k/q project
    first, so the first exps start ~25us in; remaining projections fill
    PE slack under the first units' exps (later q chunks stay lazy as
    mid-attention PE gap filler)
  - attention per head-pair x 512-query chunk: S matmuls packed two
    heads per pass via disjoint PE row groups into one [128,1024] PSUM
    tile, one wide exp on ACT (scale=1/8, fp16 out), PV accumulation
  - normalization: denominator copy + reciprocal_approx_fast issue ahead
    of the bulk evacuation (DVE), partition_broadcast on GPSIMD, one DVE
    multiply -- PE and ACT stay out of the chain; emission lags two units
    behind compute; the final unit normalizes straight from PSUM
  - out-projection partial [1024,2048] per chunk in fp16, staged via DVE
    (ACT only runs exps); the last chunk's out-projection routes through
    the then-free S-score PSUM banks so its chains pre-run during the
    final softmax normalization
Host sums the two head-group partials per batch in fp32 and adds bias.
Measured: ~387us per core on trn2 (baseline 528us), absmax rel err 8e-4.
"""
from collections import deque
from contextlib import ExitStack

import numpy as np

import concourse.mybir as mybir
import concourse.tile as tile
from concourse import bacc
from concourse.bass_utils import run_bass_kernel_spmd

F32 = mybir.dt.float32
F16 = mybir.dt.float16

P = 128
N = 2048         # sequence length
DI = 1024        # model dim
NH = 8           # heads per core
HD = 64          # head dim
NPAIR = 4        # head pairs per core
KT = 8           # contraction tiles for projections
CH = 512         # query chunk width
NCHUNK = 4       # chunks per sequence
MT = 16          # key tiles (m) per sequence
ET = 8           # output-feature blocks
SCALE = HD ** -0.5

_NC_CACHE = None


def _build():
    nc = bacc.Bacc("TRN2", target_bir_lowering=False, debug=False)

    xT = nc.dram_tensor("xT", [KT, NCHUNK, P, CH], F16,
                        kind="ExternalInput").ap()
    wqkA = nc.dram_tensor("wqkA", [8, P, KT, P], F16, kind="ExternalInput").ap()
    wvA = nc.dram_tensor("wvA", [P, KT, 512], F16, kind="ExternalInput").ap()
    woT = nc.dram_tensor("woT", [512, DI], F16, kind="ExternalInput").ap()
    outT = nc.dram_tensor("outT", [DI, N], F16, kind="ExternalOutput").ap()

    woT_r = woT.rearrange("(k p) e -> k p e", p=P)      # [4, 128, 1024]
    outT_r = outT.rearrange("(e p) n -> e p n", p=P)    # [8, 128, 2048]

    with tile.TileContext(nc) as tc, ExitStack() as persist:
        qk_pool = persist.enter_context(tc.tile_pool(name="qkp", bufs=8))
        va_pool = persist.enter_context(tc.tile_pool(name="vap", bufs=1))
        misc = persist.enter_context(tc.tile_pool(name="misc", bufs=1))
        xt_pool = persist.enter_context(tc.tile_pool(name="xt", bufs=8))
        wq_pool = persist.enter_context(tc.tile_pool(name="wq", bufs=4))
        wv_pool = persist.enter_context(tc.tile_pool(name="wv", bufs=1))
        wo_pool = persist.enter_context(tc.tile_pool(name="wo", bufs=4))
        exp_pool = persist.enter_context(tc.tile_pool(name="expp", bufs=17))
        ot_pool = persist.enter_context(tc.tile_pool(name="ot", bufs=8))
        osb_pool = persist.enter_context(tc.tile_pool(name="osb", bufs=8))
        stage_pool = persist.enter_context(tc.tile_pool(name="stg", bufs=3))
        rden_pool = persist.enter_context(tc.tile_pool(name="rden", bufs=12))
        stg2_pool = persist.enter_context(tc.tile_pool(name="stg2", bufs=2))
        rbc_pool = persist.enter_context(tc.tile_pool(name="rbc", bufs=4))
        sps_pool = persist.enter_context(
            tc.tile_pool(name="sps", bufs=2, space="PSUM"))
        oaug_pool = persist.enter_context(
            tc.tile_pool(name="oaug", bufs=2, space="PSUM"))
        aux_pool = persist.enter_context(
            tc.tile_pool(name="aux", bufs=2, space="PSUM"))

        # ---- DMAs in priority order: pair-0 weights first, then x ----
        # xT arrives in per-chunk column slices so the first kproj/qproj
        # chains (and with them the first exps) start ~6us in instead of
        # waiting for the whole 4MB transfer
        wqk_k = [None] * 8
        wqk_k[4] = wq_pool.tile([P, KT, P], F16, name="wk4", tag="wk")
        nc.sync.dma_start(wqk_k[4][:], wqkA[4])
        xt = [xt_pool.tile([P, N], F16, name=f"xt{k}", tag="xt")
              for k in range(KT)]
        for k in range(KT):
            nc.sync.dma_start(xt[k][:, 0:CH], xT[k, 0])
        wq = [wq_pool.tile([P, KT, P], F16, name=f"wq{f}", tag="wq")
              for f in range(4)]
        nc.sync.dma_start(wq[0][:], wqkA[0])
        for k in range(KT):
            nc.sync.dma_start(xt[k][:, CH:2 * CH], xT[k, 1])
        nc.sync.dma_start(wq[1][:], wqkA[1])
        wqk_k[5] = wq_pool.tile([P, KT, P], F16, name="wk5", tag="wk")
        nc.sync.dma_start(wqk_k[5][:], wqkA[5])
        nc.sync.dma_start(wq[2][:], wqkA[2])
        nc.sync.dma_start(wq[3][:], wqkA[3])
        for c in (2, 3):
            for k in range(KT):
                nc.sync.dma_start(xt[k][:, c * CH:(c + 1) * CH], xT[k, c])
        for f in (6, 7):
            wqk_k[f] = wq_pool.tile([P, KT, P], F16, name=f"wk{f}", tag="wk")
            nc.sync.dma_start(wqk_k[f][:], wqkA[f])
        wv = wv_pool.tile([P, KT, 512], F16)
        nc.sync.dma_start(wv[:], wvA[:])
        wo = [wo_pool.tile([P, DI], F16, name=f"wo{kk}", tag="wo")
              for kk in range(NPAIR)]
        for kk in range(NPAIR):
            nc.sync.dma_start(wo[kk][:], woT_r[kk])

        # qkT tiles: 0..3 = q head-pairs, 4..7 = k head-pairs.
        # Tile j holds heads 2j (parts 0:64) and 2j+1 (parts 64:128).
        qkT = [qk_pool.tile([P, N], F16, name=f"qkT{t}", tag="qkT")
               for t in range(8)]
        v_aug = va_pool.tile([P, MT, NH, HD + 1], F16)
        nc.vector.memset(v_aug[:, :, :, HD:HD + 1], 1.0)

        # ---------------- projection emitters --------------------------
        def emit_kproj(f, chunks=tuple(range(NCHUNK))):
            for c in chunks:
                ps = aux_pool.tile([P, CH], F32, tag="aux", name=f"kp_{f}_{c}")
                for k in range(KT):
                    nc.tensor.matmul(ps[:], wqk_k[f][:, k, :],
                                     xt[k][:, c * CH:(c + 1) * CH],
                                     start=(k == 0), stop=(k == KT - 1))
                nc.vector.tensor_copy(qkT[f][:, c * CH:(c + 1) * CH], ps[:])

        def emit_vproj_row(r):
            ps = aux_pool.tile([P, CH], F32, tag="aux", name=f"vp_{r}")
            for k in range(KT):
                nc.tensor.matmul(ps[:], xt[k][:, r * P:(r + 1) * P],
                                 wv[:, k, :],
                                 start=(k == 0), stop=(k == KT - 1))
            nc.vector.tensor_copy(v_aug[:, r, :, 0:HD],
                                  ps.rearrange("p (h d) -> p h d", d=HD))

        def emit_qproj(c, f):
            csl = slice(c * CH, (c + 1) * CH)
            ps = aux_pool.tile([P, CH], F32, tag="aux", name=f"qp_{c}_{f}")
            for k in range(KT):
                nc.tensor.matmul(ps[:], wq[f][:, k, :], xt[k][:, csl],
                                 start=(k == 0), stop=(k == KT - 1))
            nc.vector.tensor_copy(qkT[f][:, csl], ps[:])

        # ---------------- attention emitters ---------------------------
        ot_map = {}

        def emit_sphase(c, p):
            csl = slice(c * CH, (c + 1) * CH)
            qA = qkT[p][0:HD, csl]
            qB = qkT[p][HD:P, csl]
            kTl = qkT[4 + p]
            expPs = []
            for m in range(MT):
                msl = slice(m * P, (m + 1) * P)
                s_ps = sps_pool.tile([P, 2 * CH], F32, tag="sps",
                                     name=f"sps_{c}_{p}_{m}")
                # packed S matmuls: head A rows 0:64, head B rows 64:128
                # (disjoint PE row groups, run concurrently)
                nc.tensor.matmul(s_ps[:, 0:CH], kTl[0:HD, msl], qA,
                                 start=True, stop=True)
                nc.tensor.matmul(s_ps[:, CH:2 * CH], kTl[HD:P, msl], qB,
                                 start=True, stop=True)
                expP = exp_pool.tile([P, 2 * CH], F16, tag="expp",
                                     name=f"expP_{c}_{p}_{m}")
                nc.scalar.activation(expP[:], s_ps[:],
                                     mybir.ActivationFunctionType.Exp,
                                     scale=SCALE)
                expPs.append(expP)
            return expPs

        def emit_pvphase(c, p, expPs, per_m=None):
            oaugA = oaug_pool.tile([P, CH], F32, tag="oaug",
                                   name=f"oaugA_{c}_{p}")
            oaugB = oaug_pool.tile([P, CH], F32, tag="oaug",
                                   name=f"oaugB_{c}_{p}")
            for m in range(MT):
                if per_m is not None:
                    per_m(m)
                nc.tensor.matmul(oaugA[0:HD + 1, :],
                                 v_aug[:, m, 2 * p, :],
                                 expPs[m][:, 0:CH],
                                 start=(m == 0), stop=(m == MT - 1))
                nc.tensor.matmul(oaugB[0:HD + 1, :],
                                 v_aug[:, m, 2 * p + 1, :],
                                 expPs[m][:, CH:2 * CH],
                                 start=(m == 0), stop=(m == MT - 1))

            # evacuate + fast reciprocal of the raw [1,512] denominator row
            # (den copied to a partition-0 tile: reciprocal_approx_fast
            # cannot read from a partition-offset AP)
            o_sbs = []
            last = (c, p) == (NCHUNK - 1, NPAIR - 1)
            for half, oaug in ((0, oaugA), (1, oaugB)):
                den = rden_pool.tile([1, CH], F32, tag="rden",
                                     name=f"den_{c}_{p}_{half}")
                nc.vector.tensor_copy(den[:], oaug[HD:HD + 1, :])
                rden = rden_pool.tile([1, CH], F32, tag="rden",
                                      name=f"rden_{c}_{p}_{half}")
                nc.vector.reciprocal_approx_fast(rden[:], den[:])
                if last:
                    # tail: normalize straight from the PSUM bank
                    o_sb = oaug
                else:
                    o_sb = osb_pool.tile([HD, CH], F16, tag="osb",
                                         name=f"osb_{c}_{p}_{half}")
                    nc.vector.tensor_copy(o_sb[:], oaug[0:HD, :])
                o_sbs.append((o_sb, rden))
            return (c, p, o_sbs)

        def emit_unit(c, p):
            return emit_pvphase(c, p, emit_sphase(c, p))

        def emit_norm(unit):
            # broadcast the reciprocal across partitions on GPSIMD, then
            # one DVE multiply; emitted one unit late so nothing waits
            c, p, o_sbs = unit
            ot_p = ot_pool.tile([P, CH], F16, name=f"ot_{c}_{p}", tag="ot")
            for half, (o_sb, rden) in ((0, o_sbs[0]), (1, o_sbs[1])):
                rbc = rbc_pool.tile([HD, CH], F32, tag="rbc",
                                    name=f"rbc_{c}_{p}_{half}")
                nc.gpsimd.partition_broadcast(rbc[:], rden[:], channels=HD)
                nc.vector.tensor_tensor(
                    ot_p[half * HD:(half + 1) * HD, :],
                    o_sb[0:HD, :], rbc[:], mybir.AluOpType.mult)
            ot_map[(c, p)] = ot_p

        def emit_outproj(c, es=tuple(range(ET))):
            csl = slice(c * CH, (c + 1) * CH)
            for e in es:
                pso = aux_pool.tile([P, CH], F32, tag="aux",
                                    name=f"pso_{c}_{e}")
                for p in range(NPAIR):
                    nc.tensor.matmul(pso[:],
                                     wo[p][:, e * P:(e + 1) * P],
                                     ot_map[(c, p)][:],
                                     start=(p == 0), stop=(p == NPAIR - 1))
                st = stage_pool.tile([P, CH], F16, tag="stg",
                                     name=f"st_{c}_{e}")
                nc.vector.tensor_copy(st[:], pso[:])
                nc.sync.dma_start(outT_r[e][:, csl], st[:])

        def emit_outproj_final(c):
            # last chunk: the sps banks are free (no exps left) — pack two
            # e-blocks per [128,1024] PSUM tile so four chains pre-run
            # their p<3 matmuls while the last norm is still in flight
            csl = slice(c * CH, (c + 1) * CH)
            for ep in range(ET // 2):
                pso = sps_pool.tile([P, 2 * CH], F32, tag="sps",
                                    name=f"psof_{ep}")
                for half in (0, 1):
                    e = 2 * ep + half
                    for p in range(NPAIR):
                        nc.tensor.matmul(pso[:, half * CH:(half + 1) * CH],
                                         wo[p][:, e * P:(e + 1) * P],
                                         ot_map[(c, p)][:],
                                         start=(p == 0),
                                         stop=(p == NPAIR - 1))
                st = stg2_pool.tile([P, 2 * CH], F16, tag="stg2",
                                    name=f"stf_{ep}")
                nc.vector.tensor_copy(st[:, 0:CH], pso[:, 0:CH])
                nc.sync.dma_start(outT_r[2 * ep][:, csl], st[:, 0:CH])
                nc.vector.tensor_copy(st[:, CH:2 * CH], pso[:, CH:2 * CH])
                nc.scalar.dma_start(outT_r[2 * ep + 1][:, csl],
                                    st[:, CH:2 * CH])

        # ---------------- emission schedule -----------------------------
        # Pair-0 projections first so unit (0,0) unblocks ~35us in; the
        # rest of the projections fill PE slack under the first exps.
        # Software pipeline: the norm for unit i is emitted after unit
        # i+1's matmuls, and chunk c's out-projection after chunk c+1's
        # second unit, so the PE never waits on the DVE chain.
        pend = deque()
        # chunk-0 ramp-up: pair-0's S-phase starts as soon as kT4-chunk0
        # and q pair-0 land; vproj rows interleave with PV(0,0) matmuls
        # (PV(0,0,m) must follow vproj row m in program order) so the PE
        # alternates instead of draining all of vproj before any PV
        emit_kproj(4, chunks=(0,))
        emit_qproj(0, 0)
        emit_qproj(0, 1)
        emit_kproj(4, chunks=(1, 2, 3))
        sp00 = emit_sphase(0, 0)
        emit_kproj(5, chunks=(0,))
        emit_kproj(5, chunks=(1, 2, 3))
        sp01 = emit_sphase(0, 1)
        for r in range(MT):
            emit_vproj_row(r)
        pend.append(emit_pvphase(0, 0, sp00))
        emit_qproj(0, 2)
        emit_qproj(0, 3)
        pend.append(emit_pvphase(0, 1, sp01))
        emit_qproj(1, 0)
        emit_qproj(1, 1)
        emit_kproj(6)
        emit_kproj(7)
        emit_qproj(1, 2)
        emit_qproj(1, 3)
        for p in (2, 3):
            pend.append(emit_unit(0, p))
            if len(pend) > 2:
                emit_norm(pend.popleft())
        for c in range(1, NCHUNK):
            for p in range(NPAIR):
                pend.append(emit_unit(c, p))
                lag = 1 if c == NCHUNK - 1 else 2
                while len(pend) > lag:
                    emit_norm(pend.popleft())
                if c + 1 < NCHUNK and p in (0, 1):
                    emit_qproj(c + 1, 2 * p)
                    emit_qproj(c + 1, 2 * p + 1)
                if p == 2:
                    while pend and pend[0][0] < c:
                        emit_norm(pend.popleft())
                    emit_outproj(c - 1, es=(0, 1, 2, 3))
                if p == 3:
                    emit_outproj(c - 1, es=(4, 5, 6, 7))
        while pend:
            emit_norm(pend.popleft())
        emit_outproj_final(NCHUNK - 1)

    nc.compile()
    return nc


def _get_nc():
    global _NC_CACHE
    if _NC_CACHE is None:
        _NC_CACHE = _build()
    return _NC_CACHE


def _make_in_maps(x, w_qkv, w_out):
    per_g = []
    for g in range(2):
        qk_g = np.concatenate([w_qkv[g * 512:(g + 1) * 512],
                               w_qkv[DI + g * 512:DI + (g + 1) * 512]], axis=0)
        wqkT = np.ascontiguousarray(qk_g.T)               # [1024 d, 1024 f]
        wqkA = np.ascontiguousarray(
            wqkT.reshape(KT, P, 8, P).transpose(2, 1, 0, 3).astype(np.float16))
        v_g = w_qkv[2 * DI + g * 512:2 * DI + (g + 1) * 512]
        wvT = np.ascontiguousarray(v_g.T)                 # [1024 d, 512 f]
        wvA = np.ascontiguousarray(
            wvT.reshape(KT, P, 512).transpose(1, 0, 2).astype(np.float16))
        woTg = np.ascontiguousarray(
            w_out[:, g * 512:(g + 1) * 512].T.astype(np.float16))
        per_g.append((wqkA, wvA, woTg))

    in_maps = []
    for c in range(8):
        b, g = c // 2, c % 2
        wqkA, wvA, woTg = per_g[g]
        in_maps.append({
            "xT": np.ascontiguousarray(
                x[b].T.astype(np.float16).reshape(KT, P, NCHUNK, CH)
                .transpose(0, 2, 1, 3)),
            "wqkA": wqkA,
            "wvA": wvA,
            "woT": woTg,
        })
    return in_maps


def kernel(x, w_qkv, w_out, b_out):
    x = np.asarray(x, dtype=np.float32)
    w_qkv = np.asarray(w_qkv, dtype=np.float32)
    w_out = np.asarray(w_out, dtype=np.float32)
    b_out = np.asarray(b_out, dtype=np.float32)
    B = x.shape[0]

    in_maps = _make_in_maps(x, w_qkv, w_out)
    nc = _get_nc()
    res = run_bass_kernel_spmd(nc, in_maps, core_ids=list(range(8)))
    parts = [r["outT"] for r in res.results]
    out = np.empty((B, N, DI), dtype=np.float32)
    for b in range(B):
        out[b] = (parts[2 * b].astype(np.float32)
                  + parts[2 * b + 1].astype(np.float32)).T + b_out
    return out
